# revision 17
# baseline (speedup 1.0000x reference)
"""Trainium2 Bass kernel for DenseMLPQMatrixDecoder.

Math: per embedding v, a tiny MLP (8->16->16->16) produces logits for a 4x4
rate matrix Q (zero diag -> exp -> row-normalize off-diag -> diag = -1).
The reference then computes expm(Q*1000) per (v, s) and takes row 0.

Key facts (verified against the reference numerically):
  * site_positions is never used numerically -- the S axis is a pure
    broadcast of the per-v result.
  * The slowest-mixing Q over the input distribution has spectral gap
    ~1.1, so expm(Q*1000) == the stationary distribution pi of Q to well
    below float32 resolution.  pi is computed exactly via the Markov-chain
    tree theorem: pi_i proportional to the (i,i) principal minor of Q
    (all four minors share one sign, so normalization cancels it).

Sharding: V=1024 split as 128 rows per core across 8 cores (pure data
parallel); MLP weights replicated.  Each core computes pi for its 128 v's
([128,4]), replicates along the free dim to [128, S*4], and writes its
contiguous 2MB slice of the output.

Hardware constraints honored (trn2 walrus codegen):
  * PE Matmult / ACT Activation instructions can carry only ONE sync wait,
    so every matmul input is produced by the ACT engine (single semaphore)
    and activation biases are read from the DMA-raw tile whose semaphore
    ACT observed at its first copy.
  * The kernel-tail Drain waits once per logical processor used, and its
    wait budget is small -- the kernel uses only ACT, PE, DVE and two DMA
    queues (all inputs ride ONE dma: weights, biases and the pre-transposed
    embedding shard are host-packed into a single [17, 179] tensor).
"""

import numpy as np

import concourse.bacc as bacc
import concourse.bass as bass
import concourse.mybir as mybir
import concourse.tile as tile
from concourse.bass_utils import run_bass_kernel_spmd

AF = mybir.ActivationFunctionType
F32 = mybir.dt.float32

V, D, WIDTH, A, S = 1024, 8, 16, 4, 1024
NCORES = 8
VP = V // NCORES          # 128 rows per core
FREE = S * A              # 4096 output elems per row

# Packed input layout [PACK_P, PACK_F] f32 (one DMA -> one semaphore):
#   rows 0:16 cols  0:17   W1 padded with a zero 17th column (so the mm2
#                          output row 16 is 0; relu(0 + bias 1.0) = 1 builds
#                          the ones-row for the augmented-bias last layer)
#   rows 0:17 cols 17:33   Wout with bout as row 16
#   rows 0:16 col  33      b0
#   rows 0:17 col  34      b1 with 1.0 at row 16
#   rows 0:8  cols 35:51   W0
#   rows 0:8  cols 51:179  emb shard, pre-transposed to [D, VP]
PACK_P = WIDTH + 1        # 17
PACK_F = 51 + VP          # 179


def pack_inputs(W0, b0, W1, b1, Wout, bout, emb) -> list[np.ndarray]:
    """Per-core packed input tensors (emb: full [V, D] array)."""
    base = np.zeros((PACK_P, PACK_F), np.float32)
    base[0:WIDTH, 0:16] = W1
    base[0:WIDTH, 17:33] = Wout
    base[WIDTH, 17:33] = bout
    base[0:WIDTH, 33] = b0
    base[0:WIDTH, 34] = b1
    base[WIDTH, 34] = 1.0
    base[0:D, 35:51] = W0
    packs = []
    for c in range(NCORES):
        p = base.copy()
        p[0:D, 51 : 51 + VP] = emb[c * VP : (c + 1) * VP].T
        packs.append(p)
    return packs


def _diag_ap(t):
    """AP selecting the 4 diagonal elements (free offsets 0,5,10,15)."""
    ap = t[:]
    return bass.AP(tensor=ap.tensor, offset=ap.offset, ap=[ap.ap[0], [5, 4]])


def _build_module() -> bass.Bass:
    # Bacc (not plain Bass): its compile()/finalize() pipeline legalizes
    # multi-wait instructions (move_matmul_waits_to_ldweights +
    # generate_event_semaphores) for the TRN2 1-wait-per-instruction limit.
    nc = bacc.Bacc()

    pack_d = nc.declare_dram_parameter("pack", [PACK_P, PACK_F], F32, isOutput=False)
    out_d = nc.declare_dram_parameter("out", [VP, FREE], F32, isOutput=True)

    with tile.TileContext(nc) as tc:
        with (
            tc.tile_pool(name="sb", bufs=1) as sb,
            tc.tile_pool(name="ps", bufs=1, space="PSUM") as ps,
        ):
            # ---- load everything with ONE dma --------------------------------
            raw = sb.tile([PACK_P, PACK_F], F32)
            nc.sync.dma_start(raw[:], pack_d[:])

            w1_aug = raw[0:WIDTH, 0:17]         # [16,17], col 16 = zeros
            wout_aug = raw[0:PACK_P, 17:33]     # [17,16], row 16 = bout
            w0_sb = raw[0:D, 35:51]
            embT = raw[0:D, 51 : 51 + VP]       # [8,128]
            b0_raw = raw[0:WIDTH, 33:34]
            b1_raw = raw[0:PACK_P, 34:35]

            # ---- MLP in feature-major layout: h_T = W.T @ x_T ----------------
            ps1 = ps.tile([WIDTH, VP], F32)
            nc.tensor.matmul(ps1[:], w0_sb, embT)
            h0 = sb.tile([WIDTH, VP], F32)
            nc.scalar.activation(h0[:], ps1[:], AF.Relu, bias=b0_raw)

            ps2 = ps.tile([PACK_P, VP], F32)    # [17,128]; row 16 = 0 (zero W1 col)
            nc.tensor.matmul(ps2[:], w1_aug, h0[:])
            h1a = sb.tile([PACK_P, VP], F32)    # [17,128]: relu rows + ones row 16
            nc.scalar.activation(h1a[:], ps2[:], AF.Relu, bias=b1_raw)

            # last layer emitted v-major directly: logq[v,k] = h1a.T @ Wout_aug
            ps3 = ps.tile([VP, A * A], F32)
            nc.tensor.matmul(ps3[:], h1a[:], wout_aug)
            e = sb.tile([VP, A * A], F32)       # E = exp(logq + bout)
            nc.scalar.activation(e[:], ps3[:], AF.Exp)

            # ---- build Q: zero diag, row-normalize, diag=-1 ------------------
            nc.vector.memset(_diag_ap(e), 0.0)
            r = sb.tile([VP, A], F32)
            nc.vector.reduce_sum(
                r[:], e[:].rearrange("p (i j) -> p i j", i=A), axis=mybir.AxisListType.X
            )
            rinv = sb.tile([VP, A], F32)
            nc.vector.reciprocal(rinv[:], r[:])
            q = sb.tile([VP, A * A], F32)
            nc.vector.tensor_tensor(
                q[:].rearrange("p (i j) -> p i j", i=A),
                e[:].rearrange("p (i j) -> p i j", i=A),
                rinv[:].unsqueeze(-1).broadcast_to((VP, A, A)),
                op=mybir.AluOpType.mult,
            )
            nc.vector.memset(_diag_ap(q), -1.0)

            # ---- antisymmetric 2x2 minors for row pairs (2,3),(1,3),(1,2) ----
            # M_ab[c,d] = q[a,c]*q[b,d] - q[a,d]*q[b,c]
            minors = {}
            for (a, b) in [(2, 3), (1, 3), (1, 2)]:
                outer = sb.tile([VP, A * A], F32, tag=f"outer{a}{b}")
                nc.vector.tensor_tensor(
                    outer[:].rearrange("p (c d) -> p c d", c=A),
                    q[:, 4 * a : 4 * a + 4].unsqueeze(-1).broadcast_to((VP, A, A)),
                    q[:, 4 * b : 4 * b + 4].unsqueeze(1).broadcast_to((VP, A, A)),
                    op=mybir.AluOpType.mult,
                )
                m = sb.tile([VP, A * A], F32, tag=f"m{a}{b}")
                nc.vector.tensor_tensor(
                    m[:].rearrange("p (c d) -> p c d", c=A),
                    outer[:].rearrange("p (c d) -> p c d", c=A),
                    outer[:].rearrange("p (c d) -> p d c", c=A),
                    op=mybir.AluOpType.subtract,
                )
                minors[(a, b)] = m

            # ---- principal 3x3 minors via cyclic Laplace expansion -----------
            # w_i = det of Q with row/col i removed, expanded along row e:
            #   w_i = q[e,c1]*M[c2,c3] + q[e,c2]*M[c3,c1] + q[e,c3]*M[c1,c2]
            dets = [
                (1, (1, 2, 3), minors[(2, 3)]),  # i=0
                (0, (0, 2, 3), minors[(2, 3)]),  # i=1
                (0, (0, 1, 3), minors[(1, 3)]),  # i=2
                (0, (0, 1, 2), minors[(1, 2)]),  # i=3
            ]
            w = sb.tile([VP, A], F32)
            t0 = sb.tile([VP, 1], F32, tag="t0")
            for i, (e_row, (c1, c2, c3), m) in enumerate(dets):
                terms = [(c1, (c2, c3)), (c2, (c3, c1)), (c3, (c1, c2))]
                for k, (qc, (mc, md)) in enumerate(terms):
                    qo = 4 * e_row + qc
                    mo = 4 * mc + md
                    if k == 0:
                        nc.vector.tensor_tensor(
                            t0[:],
                            q[:, qo : qo + 1],
                            m[:, mo : mo + 1],
                            op=mybir.AluOpType.mult,
                        )
                    else:
                        # fused MAC: dst = m[mo]*q[qo] + t0
                        dst_ap = w[:, i : i + 1] if k == 2 else t0[:]
                        nc.vector.scalar_tensor_tensor(
                            dst_ap,
                            m[:, mo : mo + 1],
                            q[:, qo : qo + 1],
                            t0[:],
                            op0=mybir.AluOpType.mult,
                            op1=mybir.AluOpType.add,
                        )

            # ---- normalize: pi = w / sum(w) (sign cancels) -------------------
            wsum = sb.tile([VP, 1], F32)
            nc.vector.reduce_sum(wsum[:], w[:], axis=mybir.AxisListType.X)
            winv = sb.tile([VP, 1], F32)
            nc.vector.reciprocal(winv[:], wsum[:])
            pi = sb.tile([VP, A], F32)
            nc.vector.tensor_tensor(
                pi[:], w[:], winv[:].broadcast_to((VP, A)), op=mybir.AluOpType.mult
            )

            # ---- replicate [128,4] -> [128,512], store with repeat-source DMA
            # (2KB inner runs keep the DMA HBM-bound; no need to materialize
            # the full 16KB/partition broadcast in SBUF)
            REPW = 512
            rep = sb.tile([VP, REPW], F32)
            nc.vector.tensor_copy(rep[:, 0:A], pi[:])
            width = A
            while width < REPW:
                nc.vector.tensor_copy(rep[:, width : 2 * width], rep[:, 0:width])
                width *= 2
            nreps = FREE // REPW
            nc.sync.dma_start(
                out_d[:].rearrange("v (r f) -> v r f", r=nreps),
                rep[:].unsqueeze(1).broadcast_to((VP, nreps, REPW)),
            )

    nc.finalize()
    return nc


_NC_CACHE = None


def _get_module():
    global _NC_CACHE
    if _NC_CACHE is None:
        _NC_CACHE = _build_module()
    return _NC_CACHE


def kernel(**inputs) -> np.ndarray:
    emb = np.ascontiguousarray(np.asarray(inputs["embeddings_VxD"], np.float32))
    packs = pack_inputs(
        *[np.asarray(inputs[k], np.float32) for k in ["W0", "b0", "W1", "b1", "Wout", "bout"]],
        emb,
    )
    nc = _get_module()
    in_maps = [{"pack": packs[c]} for c in range(NCORES)]
    res = run_bass_kernel_spmd(nc, in_maps, list(range(NCORES)))
    out = np.concatenate(
        [res.results[c]["out"].reshape(VP, S, A) for c in range(NCORES)], axis=0
    )
    return out


# revision 19
# speedup vs baseline: 1.0489x; 1.0489x over previous
"""Trainium2 Bass kernel for DenseMLPQMatrixDecoder.

Math: per embedding v, a tiny MLP (8->16->16->16) produces logits for a 4x4
rate matrix Q (zero diag -> exp -> row-normalize off-diag -> diag = -1).
The reference then computes expm(Q*1000) per (v, s) and takes row 0.

Key facts (verified against the reference numerically):
  * site_positions is never used numerically -- the S axis is a pure
    broadcast of the per-v result.
  * The slowest-mixing Q over the input distribution has spectral gap
    ~1.1, so expm(Q*1000) == the stationary distribution pi of Q to well
    below float32 resolution.  pi is computed exactly via the Markov-chain
    tree theorem: pi_i proportional to the (i,i) principal minor of Q
    (all four minors share one sign, so normalization cancels it).

Sharding: V=1024 split as 128 rows per core across 8 cores (pure data
parallel); MLP weights replicated.  Each core computes pi for its 128 v's
([128,4]), replicates along the free dim to [128, S*4], and writes its
contiguous 2MB slice of the output.

Hardware constraints honored (trn2 walrus codegen):
  * PE Matmult / ACT Activation instructions can carry only ONE sync wait,
    so every matmul input is produced by the ACT engine (single semaphore)
    and activation biases are read from the DMA-raw tile whose semaphore
    ACT observed at its first copy.
  * The kernel-tail Drain waits once per logical processor used, and its
    wait budget is small -- the kernel uses only ACT, PE, DVE and two DMA
    queues (all inputs ride ONE dma: weights, biases and the pre-transposed
    embedding shard are host-packed into a single [17, 179] tensor).
"""

import numpy as np

import concourse.bacc as bacc
import concourse.bass as bass
import concourse.mybir as mybir
import concourse.tile as tile
from concourse.bass_utils import run_bass_kernel_spmd

AF = mybir.ActivationFunctionType
F32 = mybir.dt.float32

V, D, WIDTH, A, S = 1024, 8, 16, 4, 1024
NCORES = 8
VP = V // NCORES          # 128 rows per core
FREE = S * A              # 4096 output elems per row

# Packed input layout [PACK_P, PACK_F] f32 (one DMA -> one semaphore):
#   rows 0:16 cols  0:17   W1 padded with a zero 17th column (so the mm2
#                          output row 16 is 0; relu(0 + bias 1.0) = 1 builds
#                          the ones-row for the augmented-bias last layer)
#   rows 0:17 cols 17:33   Wout with bout as row 16
#   rows 0:16 col  33      b0
#   rows 0:17 col  34      b1 with 1.0 at row 16
#   rows 0:8  cols 35:51   W0
#   rows 0:8  cols 51:179  emb shard, pre-transposed to [D, VP]
PACK_P = WIDTH + 1        # 17
PACK_F = 51 + VP          # 179


def pack_inputs(W0, b0, W1, b1, Wout, bout, emb) -> list[np.ndarray]:
    """Per-core packed input tensors (emb: full [V, D] array)."""
    base = np.zeros((PACK_P, PACK_F), np.float32)
    base[0:WIDTH, 0:16] = W1
    base[0:WIDTH, 17:33] = Wout
    base[WIDTH, 17:33] = bout
    base[0:WIDTH, 33] = b0
    base[0:WIDTH, 34] = b1
    base[WIDTH, 34] = 1.0
    base[0:D, 35:51] = W0
    packs = []
    for c in range(NCORES):
        p = base.copy()
        p[0:D, 51 : 51 + VP] = emb[c * VP : (c + 1) * VP].T
        packs.append(p)
    return packs


def _diag_ap(t):
    """AP selecting the 4 diagonal elements (free offsets 0,5,10,15)."""
    ap = t[:]
    return bass.AP(tensor=ap.tensor, offset=ap.offset, ap=[ap.ap[0], [5, 4]])


def _build_module() -> bass.Bass:
    # Bacc (not plain Bass): its compile()/finalize() pipeline legalizes
    # multi-wait instructions (move_matmul_waits_to_ldweights +
    # generate_event_semaphores) for the TRN2 1-wait-per-instruction limit.
    nc = bacc.Bacc()

    pack_d = nc.declare_dram_parameter("pack", [PACK_P, PACK_F], F32, isOutput=False)
    out_d = nc.declare_dram_parameter("out", [VP, FREE], F32, isOutput=True)

    with tile.TileContext(nc) as tc:
        with (
            tc.tile_pool(name="sb", bufs=1) as sb,
            tc.tile_pool(name="ps", bufs=1, space="PSUM") as ps,
        ):
            # Dummy no-dep activation: pulls the ~1.3us ACT_TABLE_LOAD to the
            # head of the kernel (parallel with the input DMA) instead of the
            # critical path before the first real activation.
            warm = sb.tile([1, 1], F32)
            nc.scalar.activation(warm[:], nc.const_aps.tensor(0.0, (1, 1)), AF.Exp)

            # ---- load everything with ONE dma --------------------------------
            raw = sb.tile([PACK_P, PACK_F], F32)
            nc.sync.dma_start(raw[:], pack_d[:])

            w1_aug = raw[0:WIDTH, 0:17]         # [16,17], col 16 = zeros
            wout_aug = raw[0:PACK_P, 17:33]     # [17,16], row 16 = bout
            w0_sb = raw[0:D, 35:51]
            embT = raw[0:D, 51 : 51 + VP]       # [8,128]
            b0_raw = raw[0:WIDTH, 33:34]
            b1_raw = raw[0:PACK_P, 34:35]

            # ---- MLP in feature-major layout: h_T = W.T @ x_T ----------------
            ps1 = ps.tile([WIDTH, VP], F32)
            nc.tensor.matmul(ps1[:], w0_sb, embT)
            h0 = sb.tile([WIDTH, VP], F32)
            nc.scalar.activation(h0[:], ps1[:], AF.Relu, bias=b0_raw)

            ps2 = ps.tile([PACK_P, VP], F32)    # [17,128]; row 16 = 0 (zero W1 col)
            nc.tensor.matmul(ps2[:], w1_aug, h0[:])
            h1a = sb.tile([PACK_P, VP], F32)    # [17,128]: relu rows + ones row 16
            nc.scalar.activation(h1a[:], ps2[:], AF.Relu, bias=b1_raw)

            # last layer emitted v-major directly: logq[v,k] = h1a.T @ Wout_aug
            ps3 = ps.tile([VP, A * A], F32)
            nc.tensor.matmul(ps3[:], h1a[:], wout_aug)
            e = sb.tile([VP, A * A], F32)       # E = exp(logq + bout)
            nc.scalar.activation(e[:], ps3[:], AF.Exp)

            # ---- build Q: zero diag, row-normalize, diag=-1 ------------------
            nc.vector.memset(_diag_ap(e), 0.0)
            r = sb.tile([VP, A], F32)
            nc.vector.reduce_sum(
                r[:], e[:].rearrange("p (i j) -> p i j", i=A), axis=mybir.AxisListType.X
            )
            rinv = sb.tile([VP, A], F32)
            nc.vector.reciprocal(rinv[:], r[:])
            q = sb.tile([VP, A * A], F32)
            nc.vector.tensor_tensor(
                q[:].rearrange("p (i j) -> p i j", i=A),
                e[:].rearrange("p (i j) -> p i j", i=A),
                rinv[:].unsqueeze(-1).broadcast_to((VP, A, A)),
                op=mybir.AluOpType.mult,
            )
            nc.vector.memset(_diag_ap(q), -1.0)

            # ---- antisymmetric 2x2 minors for row pairs (2,3),(1,3),(1,2) ----
            # M_ab[c,d] = q[a,c]*q[b,d] - q[a,d]*q[b,c]
            minors = {}
            for (a, b) in [(2, 3), (1, 3), (1, 2)]:
                outer = sb.tile([VP, A * A], F32, tag=f"outer{a}{b}")
                nc.vector.tensor_tensor(
                    outer[:].rearrange("p (c d) -> p c d", c=A),
                    q[:, 4 * a : 4 * a + 4].unsqueeze(-1).broadcast_to((VP, A, A)),
                    q[:, 4 * b : 4 * b + 4].unsqueeze(1).broadcast_to((VP, A, A)),
                    op=mybir.AluOpType.mult,
                )
                m = sb.tile([VP, A * A], F32, tag=f"m{a}{b}")
                nc.vector.tensor_tensor(
                    m[:].rearrange("p (c d) -> p c d", c=A),
                    outer[:].rearrange("p (c d) -> p c d", c=A),
                    outer[:].rearrange("p (c d) -> p d c", c=A),
                    op=mybir.AluOpType.subtract,
                )
                minors[(a, b)] = m

            # ---- principal 3x3 minors via cyclic Laplace expansion -----------
            # w_i = det of Q with row/col i removed, expanded along row e:
            #   w_i = q[e,c1]*M[c2,c3] + q[e,c2]*M[c3,c1] + q[e,c3]*M[c1,c2]
            dets = [
                (1, (1, 2, 3), minors[(2, 3)]),  # i=0
                (0, (0, 2, 3), minors[(2, 3)]),  # i=1
                (0, (0, 1, 3), minors[(1, 3)]),  # i=2
                (0, (0, 1, 2), minors[(1, 2)]),  # i=3
            ]
            w = sb.tile([VP, A], F32)
            t0 = sb.tile([VP, 1], F32, tag="t0")
            for i, (e_row, (c1, c2, c3), m) in enumerate(dets):
                terms = [(c1, (c2, c3)), (c2, (c3, c1)), (c3, (c1, c2))]
                for k, (qc, (mc, md)) in enumerate(terms):
                    qo = 4 * e_row + qc
                    mo = 4 * mc + md
                    if k == 0:
                        nc.vector.tensor_tensor(
                            t0[:],
                            q[:, qo : qo + 1],
                            m[:, mo : mo + 1],
                            op=mybir.AluOpType.mult,
                        )
                    else:
                        # fused MAC: dst = m[mo]*q[qo] + t0
                        dst_ap = w[:, i : i + 1] if k == 2 else t0[:]
                        nc.vector.scalar_tensor_tensor(
                            dst_ap,
                            m[:, mo : mo + 1],
                            q[:, qo : qo + 1],
                            t0[:],
                            op0=mybir.AluOpType.mult,
                            op1=mybir.AluOpType.add,
                        )

            # ---- normalize: pi = w / sum(w) (sign cancels) -------------------
            wsum = sb.tile([VP, 1], F32)
            nc.vector.reduce_sum(wsum[:], w[:], axis=mybir.AxisListType.X)
            winv = sb.tile([VP, 1], F32)
            nc.vector.reciprocal(winv[:], wsum[:])
            pi = sb.tile([VP, A], F32)
            nc.vector.tensor_tensor(
                pi[:], w[:], winv[:].broadcast_to((VP, A)), op=mybir.AluOpType.mult
            )

            # ---- replicate [128,4] -> [128,512], store with repeat-source DMA
            # (2KB inner runs keep the DMA HBM-bound; no need to materialize
            # the full 16KB/partition broadcast in SBUF)
            REPW = 512
            rep = sb.tile([VP, REPW], F32)
            # single broadcast-source copy: read pi 128x with a stride-0 dim
            nc.vector.tensor_copy(
                rep[:].rearrange("p (r f) -> p r f", f=A),
                pi[:].unsqueeze(1).broadcast_to((VP, REPW // A, A)),
            )
            nreps = FREE // REPW
            nc.sync.dma_start(
                out_d[:].rearrange("v (r f) -> v r f", r=nreps),
                rep[:].unsqueeze(1).broadcast_to((VP, nreps, REPW)),
            )

    nc.finalize()
    return nc


_NC_CACHE = None


def _get_module():
    global _NC_CACHE
    if _NC_CACHE is None:
        _NC_CACHE = _build_module()
    return _NC_CACHE


def kernel(**inputs) -> np.ndarray:
    emb = np.ascontiguousarray(np.asarray(inputs["embeddings_VxD"], np.float32))
    packs = pack_inputs(
        *[np.asarray(inputs[k], np.float32) for k in ["W0", "b0", "W1", "b1", "Wout", "bout"]],
        emb,
    )
    nc = _get_module()
    in_maps = [{"pack": packs[c]} for c in range(NCORES)]
    res = run_bass_kernel_spmd(nc, in_maps, list(range(NCORES)))
    out = np.concatenate(
        [res.results[c]["out"].reshape(VP, S, A) for c in range(NCORES)], axis=0
    )
    return out


# revision 22
# speedup vs baseline: 1.0631x; 1.0136x over previous
"""Trainium2 Bass kernel for DenseMLPQMatrixDecoder.

Math: per embedding v, a tiny MLP (8->16->16->16) produces logits for a 4x4
rate matrix Q (zero diag -> exp -> row-normalize off-diag -> diag = -1).
The reference then computes expm(Q*1000) per (v, s) and takes row 0.

Key facts (verified against the reference numerically):
  * site_positions is never used numerically -- the S axis is a pure
    broadcast of the per-v result.
  * The slowest-mixing Q over the input distribution has spectral gap
    ~1.1, so expm(Q*1000) == the stationary distribution pi of Q to well
    below float32 resolution.  pi is computed exactly via the Markov-chain
    tree theorem: pi_i proportional to the (i,i) principal minor of Q
    (all four minors share one sign, so normalization cancels it).

Sharding: V=1024 split as 128 rows per core across 8 cores (pure data
parallel); MLP weights replicated.  Each core computes pi for its 128 v's
([128,4]), replicates along the free dim to [128, S*4], and writes its
contiguous 2MB slice of the output.

Hardware constraints honored (trn2 walrus codegen):
  * PE Matmult / ACT Activation instructions can carry only ONE sync wait,
    so every matmul input is produced by the ACT engine (single semaphore)
    and activation biases are read from the DMA-raw tile whose semaphore
    ACT observed at its first copy.
  * The kernel-tail Drain waits once per logical processor used, and its
    wait budget is small -- the kernel uses only ACT, PE, DVE and two DMA
    queues (all inputs ride ONE dma: weights, biases and the pre-transposed
    embedding shard are host-packed into a single [17, 179] tensor).
"""

import numpy as np

import concourse.bacc as bacc
import concourse.bass as bass
import concourse.mybir as mybir
import concourse.tile as tile
from concourse.bass_utils import run_bass_kernel_spmd

AF = mybir.ActivationFunctionType
F32 = mybir.dt.float32

V, D, WIDTH, A, S = 1024, 8, 16, 4, 1024
NCORES = 8
VP = V // NCORES          # 128 rows per core
FREE = S * A              # 4096 output elems per row

# Packed input layout [PACK_P, PACK_F] f32 (one DMA -> one semaphore):
#   rows 0:16 cols  0:17   W1 padded with a zero 17th column (so the mm2
#                          output row 16 is 0; relu(0 + bias 1.0) = 1 builds
#                          the ones-row for the augmented-bias last layer)
#   rows 0:17 cols 17:33   Wout with bout as row 16
#   rows 0:16 col  33      b0
#   rows 0:17 col  34      b1 with 1.0 at row 16
#   rows 0:8  cols 35:51   W0
#   rows 0:8  cols 51:179  emb shard, pre-transposed to [D, VP]
PACK_P = WIDTH + 1        # 17
PACK_F = 51 + VP          # 179


def pack_inputs(W0, b0, W1, b1, Wout, bout, emb) -> list[np.ndarray]:
    """Per-core packed input tensors (emb: full [V, D] array)."""
    base = np.zeros((PACK_P, PACK_F), np.float32)
    base[0:WIDTH, 0:16] = W1
    base[0:WIDTH, 17:33] = Wout
    base[WIDTH, 17:33] = bout
    base[0:WIDTH, 33] = b0
    base[0:WIDTH, 34] = b1
    base[WIDTH, 34] = 1.0
    base[0:D, 35:51] = W0
    packs = []
    for c in range(NCORES):
        p = base.copy()
        p[0:D, 51 : 51 + VP] = emb[c * VP : (c + 1) * VP].T
        packs.append(p)
    return packs


def _diag_ap(t):
    """AP selecting the 4 diagonal elements (free offsets 0,5,10,15)."""
    ap = t[:]
    return bass.AP(tensor=ap.tensor, offset=ap.offset, ap=[ap.ap[0], [5, 4]])


def _build_module() -> bass.Bass:
    # Bacc (not plain Bass): its compile()/finalize() pipeline legalizes
    # multi-wait instructions (move_matmul_waits_to_ldweights +
    # generate_event_semaphores) for the TRN2 1-wait-per-instruction limit.
    nc = bacc.Bacc()

    pack_d = nc.declare_dram_parameter("pack", [PACK_P, PACK_F], F32, isOutput=False)
    out_d = nc.declare_dram_parameter("out", [VP, FREE], F32, isOutput=True)

    with tile.TileContext(nc) as tc:
        with (
            tc.tile_pool(name="sb", bufs=1) as sb,
            tc.tile_pool(name="ps", bufs=1, space="PSUM") as ps,
        ):
            # Dummy no-dep activation: pulls the ~1.3us ACT_TABLE_LOAD to the
            # head of the kernel (parallel with the input DMA) instead of the
            # critical path before the first real activation.
            warm = sb.tile([1, 1], F32)
            nc.scalar.activation(warm[:], nc.const_aps.tensor(0.0, (1, 1)), AF.Exp)

            # ---- load everything with ONE dma --------------------------------
            raw = sb.tile([PACK_P, PACK_F], F32)
            nc.sync.dma_start(raw[:], pack_d[:])

            w1_aug = raw[0:WIDTH, 0:17]         # [16,17], col 16 = zeros
            wout_aug = raw[0:PACK_P, 17:33]     # [17,16], row 16 = bout
            w0_sb = raw[0:D, 35:51]
            embT = raw[0:D, 51 : 51 + VP]       # [8,128]
            b0_raw = raw[0:WIDTH, 33:34]
            b1_raw = raw[0:PACK_P, 34:35]

            # ---- MLP in feature-major layout: h_T = W.T @ x_T ----------------
            ps1 = ps.tile([WIDTH, VP], F32)
            nc.tensor.matmul(ps1[:], w0_sb, embT)
            h0 = sb.tile([WIDTH, VP], F32)
            nc.scalar.activation(h0[:], ps1[:], AF.Relu, bias=b0_raw)

            ps2 = ps.tile([PACK_P, VP], F32)    # [17,128]; row 16 = 0 (zero W1 col)
            nc.tensor.matmul(ps2[:], w1_aug, h0[:])
            h1a = sb.tile([PACK_P, VP], F32)    # [17,128]: relu rows + ones row 16
            nc.scalar.activation(h1a[:], ps2[:], AF.Relu, bias=b1_raw)

            # last layer emitted v-major directly: logq[v,k] = h1a.T @ Wout_aug
            ps3 = ps.tile([VP, A * A], F32)
            nc.tensor.matmul(ps3[:], h1a[:], wout_aug)
            e = sb.tile([VP, A * A], F32)       # E = exp(logq + bout)
            nc.scalar.activation(e[:], ps3[:], AF.Exp)

            # ---- build Q: zero diag, row-normalize, diag=-1 ------------------
            nc.vector.memset(_diag_ap(e), 0.0)
            r = sb.tile([VP, A], F32)
            nc.vector.reduce_sum(
                r[:], e[:].rearrange("p (i j) -> p i j", i=A), axis=mybir.AxisListType.X
            )
            rinv = sb.tile([VP, A], F32)
            nc.vector.reciprocal(rinv[:], r[:])
            q = sb.tile([VP, A * A], F32)
            nc.vector.tensor_tensor(
                q[:].rearrange("p (i j) -> p i j", i=A),
                e[:].rearrange("p (i j) -> p i j", i=A),
                rinv[:].unsqueeze(-1).broadcast_to((VP, A, A)),
                op=mybir.AluOpType.mult,
            )
            nc.vector.memset(_diag_ap(q), -1.0)

            # ---- antisymmetric 2x2 minors for row pairs (2,3),(1,3),(1,2) ----
            # M_ab[c,d] = q[a,c]*q[b,d] - q[a,d]*q[b,c]
            minors = {}
            for (a, b) in [(2, 3), (1, 3), (1, 2)]:
                outer = sb.tile([VP, A * A], F32, tag=f"outer{a}{b}")
                nc.vector.tensor_tensor(
                    outer[:].rearrange("p (c d) -> p c d", c=A),
                    q[:, 4 * a : 4 * a + 4].unsqueeze(-1).broadcast_to((VP, A, A)),
                    q[:, 4 * b : 4 * b + 4].unsqueeze(1).broadcast_to((VP, A, A)),
                    op=mybir.AluOpType.mult,
                )
                m = sb.tile([VP, A * A], F32, tag=f"m{a}{b}")
                nc.vector.tensor_tensor(
                    m[:].rearrange("p (c d) -> p c d", c=A),
                    outer[:].rearrange("p (c d) -> p c d", c=A),
                    outer[:].rearrange("p (c d) -> p d c", c=A),
                    op=mybir.AluOpType.subtract,
                )
                minors[(a, b)] = m

            # ---- principal 3x3 minors via cyclic Laplace expansion -----------
            # w_i = det of Q with row/col i removed, expanded along row e:
            #   w_i = q[e,c1]*M[c2,c3] + q[e,c2]*M[c3,c1] + q[e,c3]*M[c1,c2]
            dets = [
                (1, (1, 2, 3), minors[(2, 3)]),  # i=0
                (0, (0, 2, 3), minors[(2, 3)]),  # i=1
                (0, (0, 1, 3), minors[(1, 3)]),  # i=2
                (0, (0, 1, 2), minors[(1, 2)]),  # i=3
            ]
            w = sb.tile([VP, A], F32)
            t0 = sb.tile([VP, 1], F32, tag="t0")
            for i, (e_row, (c1, c2, c3), m) in enumerate(dets):
                terms = [(c1, (c2, c3)), (c2, (c3, c1)), (c3, (c1, c2))]
                for k, (qc, (mc, md)) in enumerate(terms):
                    qo = 4 * e_row + qc
                    mo = 4 * mc + md
                    if k == 0:
                        nc.vector.tensor_tensor(
                            t0[:],
                            q[:, qo : qo + 1],
                            m[:, mo : mo + 1],
                            op=mybir.AluOpType.mult,
                        )
                    else:
                        # fused MAC: dst = m[mo]*q[qo] + t0
                        dst_ap = w[:, i : i + 1] if k == 2 else t0[:]
                        nc.vector.scalar_tensor_tensor(
                            dst_ap,
                            m[:, mo : mo + 1],
                            q[:, qo : qo + 1],
                            t0[:],
                            op0=mybir.AluOpType.mult,
                            op1=mybir.AluOpType.add,
                        )

            # ---- normalize: pi = w / sum(w) (sign cancels) -------------------
            wsum = sb.tile([VP, 1], F32)
            nc.vector.reduce_sum(wsum[:], w[:], axis=mybir.AxisListType.X)
            winv = sb.tile([VP, 1], F32)
            nc.vector.reciprocal(winv[:], wsum[:])
            pi = sb.tile([VP, A], F32)
            nc.vector.tensor_tensor(
                pi[:], w[:], winv[:].broadcast_to((VP, A)), op=mybir.AluOpType.mult
            )

            # ---- replicate [128,4] -> [128,512], store with repeat-source DMA
            # (2KB inner runs keep the DMA HBM-bound; no need to materialize
            # the full 16KB/partition broadcast in SBUF)
            REPW = 1024
            rep = sb.tile([VP, REPW], F32)
            # single broadcast-source copy: read pi 128x with a stride-0 dim
            nc.vector.tensor_copy(
                rep[:].rearrange("p (r f) -> p r f", f=A),
                pi[:].unsqueeze(1).broadcast_to((VP, REPW // A, A)),
            )
            nreps = FREE // REPW
            nc.sync.dma_start(
                out_d[:].rearrange("v (r f) -> v r f", r=nreps),
                rep[:].unsqueeze(1).broadcast_to((VP, nreps, REPW)),
            )

    nc.finalize()
    return nc


_NC_CACHE = None


def _get_module():
    global _NC_CACHE
    if _NC_CACHE is None:
        _NC_CACHE = _build_module()
    return _NC_CACHE


def kernel(**inputs) -> np.ndarray:
    emb = np.ascontiguousarray(np.asarray(inputs["embeddings_VxD"], np.float32))
    packs = pack_inputs(
        *[np.asarray(inputs[k], np.float32) for k in ["W0", "b0", "W1", "b1", "Wout", "bout"]],
        emb,
    )
    nc = _get_module()
    in_maps = [{"pack": packs[c]} for c in range(NCORES)]
    res = run_bass_kernel_spmd(nc, in_maps, list(range(NCORES)))
    out = np.concatenate(
        [res.results[c]["out"].reshape(VP, S, A) for c in range(NCORES)], axis=0
    )
    return out
